# revision 70
# baseline (speedup 1.0000x reference)
"""Trainium2 Bass kernel for nn_Canny_61100204753382 (8-core SPMD).

Sharding: spatial row-bands (64 output rows x all 8 images per core). The
reference's flat-gather quirk reads all_filtered[k_pos, b, i, j] - the
direction index lands in the batch slot and the pixel's own batch index
selects the direction offset - so the coupling between images is at the SAME
pixel position and row-band sharding stays core-local given a small row halo.

v2 layout notes:
  - 4th input "channel" = sum of the 3 image channels (host-computed); since
    all per-channel filters are identical, conv(sum) == sum(conv), giving the
    channel-summed gradients gxs/gys with zero extra pointwise work.
  - stepA (vertical 11-tap composite convs) per (img,ch) as banded matmuls,
    PSUM tile padded to 1KB per channel so every matmul output stays inside
    one PSUM bank.
  - stepB (horizontal convs) batched 2 images/matmul in float32r (the cost
    cliff for f32r is out-free >= 256 elements).
  - phases 2/3 (NMS + hysteresis) are software-pipelined per chunk behind
    phase 1 (stepA w, stepB w-1, phase2 w-2, phase3 w-3) in DESCENDING
    chunk order so every cross-chunk boundary column (G[w+1], rs2[w+1])
    is already finished when needed; masks bf16/u8; the batched F op's
    direction axis doubles as the output-image axis (the reference's
    batch/direction swap).
  - engine split: PE convs; ACT squares/sqrt/sign/abs + some PSUM evacs;
    DVE comparisons/selects + most evacs; Pool (gpsimd, add/mult only,
    no PSUM, ~0.42 eff) takes mag/G-sum/rowsum adds except for the last
    two chunks, whose combine chain runs on DVE to shorten the tail.
    NMS comparisons must stay f32: bf16 flips ~7000 px, f32r ~930.
Host: pads & shards input rows, assembles output bands, zeroes borders.
"""

import math
import numpy as np
from contextlib import ExitStack

import concourse.bass as bass
import concourse.mybir as mybir
import concourse.tile as tile
from concourse.bass_utils import run_bass_kernel_spmd
from concourse.alu_op_type import AluOpType

f32 = mybir.dt.float32
f32r = mybir.dt.float32r
bf16 = mybir.dt.bfloat16
AF = mybir.ActivationFunctionType

B, C, H, W = 8, 3, 512, 512
C4 = 4                    # 3 channels + channel-sum
NCORES = 8
RB = H // NCORES          # output rows per core
XR = RB + 14              # input rows per core (7-row halo each side)
XC = W + 14               # padded cols
GR = RB + 4               # G rows per band (final rows -2..65)
NW = 5                    # column chunks
CW = 118                  # chunk stride (128 in-cols -> 118 out-cols)
WIN = RB + 2              # is_max row window (final rows -1..64)
T1 = float(math.tan(math.pi / 8))
T2 = float(math.tan(3 * math.pi / 8))
LOW, HIGH = 0.1, 0.3
NEIGH = [(0, 1), (1, 1), (1, 0), (1, -1), (0, -1), (-1, -1), (-1, 0), (-1, 1)]

DT = f32              # f32 = exact NMS comparisons; f32r flips ~100 px/band

_CACHE = {}
TRACE = False
LAST_EXEC_NS = None


def _band(comp, K, M, taps=11):
    Wb = np.zeros((K, M), np.float32)
    for k in range(K):
        for m in range(M):
            if 0 <= k - m < taps:
                Wb[k, m] = comp[k - m]
    return Wb


def _chunk_dims(w):
    s = CW * w
    kw = min(128, XC - s)           # in-cols this chunk
    mw = min(CW, (W + 4) - s)       # out (G) cols this chunk
    return s, kw, mw


def _build():
    nc = bass.Bass()
    for v in (-HIGH, -LOW):
        t = nc.alloc_sbuf_tensor(f"const-float32-{v}", [128, 1],
                                 mybir.dt.float32)
        nc.gpsimd.memset(t.ap(), v)
        nc.const_aps.aps[(mybir.dt.float32, v)] = t.ap()
    nc.all_engine_barrier()
    x_d = nc.dram_tensor("x", [XR, B * C4, XC], DT, kind="ExternalInput")
    wa_d = nc.dram_tensor("wa", [XR, 136], DT, kind="ExternalInput")
    wb_d = nc.dram_tensor("wb", [128, 2, 118], DT, kind="ExternalInput")
    o_d = nc.dram_tensor("o", [118, NW, B, RB], bf16, kind="ExternalOutput")

    with tile.TileContext(nc) as tc, ExitStack() as ctx:
        P = ctx.enter_context
        const = P(tc.tile_pool(name="const", bufs=1))
        big = P(tc.tile_pool(name="big", bufs=1))
        xp = P(tc.tile_pool(name="xp", bufs=2))
        sqp = P(tc.tile_pool(name="sqp", bufs=1))
        cbp = P(tc.tile_pool(name="cbp", bufs=2))
        scr = P(tc.tile_pool(name="scr", bufs=1))
        gsp = P(tc.tile_pool(name="gsp", bufs=2))
        psA = P(tc.tile_pool(name="psA", bufs=2, space="PSUM"))
        psB = P(tc.tile_pool(name="psB", bufs=2, space="PSUM"))

        wa_sb = const.tile([XR, 136], DT, tag="wa_sb")
        nc.sync.dma_start(wa_sb[:], wa_d[:])
        wb_sb = const.tile([128, 2, 118], DT, tag="wb_sb")
        nc.sync.dma_start(wb_sb[:], wb_d[:])

        def _load_x(w):
            s, kw, _ = _chunk_dims(w)
            xw = xp.tile([XR, B * C4, 128], DT, tag="xw")
            q = B * C4 // 4
            for i in range(4):
                nc.sync.dma_start(xw[:, i * q:(i + 1) * q, 0:kw],
                                  x_d[:, i * q:(i + 1) * q, s:s + kw])
            return xw

        xw_next = _load_x(NW - 1)

        gxp = P(tc.tile_pool(name="gxp", bufs=2))
        G = big.tile([128, NW, B, GR], f32, name="G", tag="G")
        gxs = big.tile([128, NW, B, GR], f32, tag="gxs")
        gys = big.tile([128, NW, B, GR], f32, tag="gys")
        u8 = mybir.dt.uint8
        hmp = P(tc.tile_pool(name="hmp", bufs=1))
        hi = big.tile([128, NW, B, WIN], bf16, tag="hi")
        mid = big.tile([128, NW, B, WIN], bf16, tag="mid")
        rs2 = big.tile([128, NW, B, RB], bf16, tag="rs2")

        # ---- phase 1: convs, magnitude, channel-summed gradients ----
        # software-pipelined: stepA(w+1) is emitted before stepB(w) so the
        # PE never stalls on the PSUM evacuations of the current chunk.
        def stepA(w, xw):
            s, kw, mw = _chunk_dims(w)
            gxA = gxp.tile([128, B, C4, 136], DT, tag="gxA")
            for img in range(B):
                pa = psA.tile([128, C4, 256], f32, tag="pa")
                for ch in range(C4):
                    nc.tensor.matmul(pa[0:kw, ch, 0:136],
                                     xw[0:XR, img * C4 + ch, 0:kw],
                                     wa_sb[0:XR], start=True, stop=True)
                if img % 4 == 3:
                    nc.scalar.copy(gxA[0:kw, img], pa[0:kw, :, 0:136])
                else:
                    nc.vector.tensor_copy(gxA[0:kw, img], pa[0:kw, :, 0:136])
            return gxA

        def stepB(w, gxA, tail=False):
            ev = nc.vector if tail else nc.gpsimd
            s, kw, mw = _chunk_dims(w)
            sq = sqp.tile([128, B, 2, C, GR], f32, tag="sq")
            for p4 in range(4):
                for j in range(2):
                    pb = psB.tile([118, 2, C, GR], f32, tag="pb")
                    nc.tensor.matmul(
                        pb[0:mw], wb_sb[0:kw, j, 0:mw],
                        gxA[0:kw, 2 * p4:2 * p4 + 2, 0:C, 68 * j:68 * j + 68],
                        start=True, stop=True)
                    nc.scalar.square(sq[0:mw, 2 * p4:2 * p4 + 2, j], pb[0:mw])
            mag = sq[:, :, 0]
            nc.vector.tensor_tensor(mag[0:118, 0:4], sq[0:118, 0:4, 0],
                                    sq[0:118, 0:4, 1], AluOpType.add)
            ev.tensor_tensor(mag[0:118, 4:8], sq[0:118, 4:8, 0],
                             sq[0:118, 4:8, 1], AluOpType.add)
            nc.scalar.sqrt(mag[0:118], mag[0:118])
            ev.tensor_tensor(G[0:118, w], mag[0:118, :, 0],
                             mag[0:118, :, 1], AluOpType.add)
            ev.tensor_tensor(G[0:118, w], G[0:118, w],
                             mag[0:118, :, 2], AluOpType.add)
            # channel-summed gradients from the xsum channel
            for j in range(2):
                dst = gxs if j == 0 else gys
                for hf in range(2):
                    pS = psB.tile([118, 4, GR], f32, tag="pS")
                    nc.tensor.matmul(
                        pS[0:mw], wb_sb[0:kw, j, 0:mw],
                        gxA[0:kw, 4 * hf:4 * hf + 4, 3, 68 * j:68 * j + 68],
                        start=True, stop=True)
                    nc.scalar.copy(dst[0:118, w, 4 * hf:4 * hf + 4],
                                   pS[0:118])

        def phase2dma(w):
            Gp = gsp.tile([128, B, GR], f32, tag="Gp")
            nc.sync.dma_start(Gp[0:117], G[1:118, w])
            if w + 1 < NW:
                nc.sync.dma_start(Gp[117:118], G[0:1, w + 1])
            Gm = gsp.tile([128, B, GR], f32, tag="Gm")
            nc.sync.dma_start(Gm[1:118], G[0:117, w])
            return Gp, Gm

        def phase2c(w, pre, tail=False):
            ev = nc.vector if tail else nc.gpsimd
            # NMS + hysteresis for one column chunk; needs G(w), G(w-1),
            # G(w+1) (boundary columns) and gxs/gys(w).
            def gw(t, dr=0):
                return t[0:118, w, :, 1 + dr:1 + dr + WIN]

            def gc(t, dr=0):
                return t[0:118, :, 1 + dr:1 + dr + WIN]

            Gp, Gm = pre
            if w > 0:
                nc.sync.dma_start(Gm[0:1], G[117:118, w - 1])
            F_all = hmp.tile([128, B, 4, WIN], bf16, tag="F_all")
            cba = cbp.tile([128, 8, B, WIN], bf16, tag="cb")
            for b in (0, 1, 7, 2, 6, 3, 4, 5):
                dr, dc = NEIGH[b]
                shs = gw(G, dr) if dc == 0 else gc({1: Gp, -1: Gm}[dc], dr)
                nc.vector.tensor_tensor(cba[0:118, b], gw(G), shs,
                                        AluOpType.is_gt)
            # F[b, h] = C_b[img h] & C_b[img h+4]; b doubles as the output
            # image index downstream (the reference's batch/direction swap)
            nc.vector.tensor_tensor(F_all[0:118], cba[0:118, :, 0:4],
                                    cba[0:118, :, 4:8], AluOpType.mult)
            # sector masks; qsm = (gxs*gys >= 0) as sign-equality
            him = hmp.tile([128, B, WIN], bf16, tag="him")
            midm = hmp.tile([128, B, WIN], bf16, tag="midm")
            qsm = hmp.tile([128, B, WIN], u8, tag="qsm")
            c1m = hmp.tile([128, B, WIN], u8, tag="c1m")
            c2m = hmp.tile([128, B, WIN], u8, tag="c2m")
            nc.scalar.sign(him[0:118], gw(gxs))
            nc.scalar.sign(midm[0:118], gw(gys))
            nc.vector.tensor_tensor(qsm[0:118], him[0:118], midm[0:118],
                                    AluOpType.is_equal)
            nc.scalar.activation(gxs[0:118, w], gxs[0:118, w], AF.Abs)
            nc.scalar.activation(gys[0:118, w], gys[0:118, w], AF.Abs)
            nc.vector.scalar_tensor_tensor(c1m[0:118], gw(gxs), T1, gw(gys),
                                           AluOpType.mult, AluOpType.is_gt)
            nc.vector.scalar_tensor_tensor(c2m[0:118], gw(gxs), T2, gw(gys),
                                           AluOpType.mult, AluOpType.is_lt)
            # 4-way select into F_all[:, 3] (pair 3 = default sector)
            sel = F_all[0:118, :, 3]
            nc.vector.copy_predicated(sel, qsm[0:118], F_all[0:118, :, 1])
            nc.vector.copy_predicated(sel, c1m[0:118], F_all[0:118, :, 0])
            nc.vector.copy_predicated(sel, c2m[0:118], F_all[0:118, :, 2])
            # hysteresis masks
            if True:
                # thresholds via ACT Sign+Relu so they run in
                # parallel with the DVE comparison chain (him = G > HIGH
                # exactly: Sign(0)=0 -> Relu 0; mm = G >= LOW differs from
                # the reference only at exact float equality with 0.1)
                nc.scalar.sign(him[0:118], gw(G), -HIGH)
                nc.scalar.activation(him[0:118], him[0:118], AF.Relu)
                mmt = hmp.tile([128, B, WIN], bf16, tag="mmt")
                nc.scalar.sign(mmt[0:118], gw(G), -LOW)
                nc.scalar.activation(mmt[0:118], mmt[0:118], AF.Relu)
                nc.gpsimd.tensor_tensor(midm[0:118], mmt[0:118],
                                        him[0:118], AluOpType.subtract)
            else:
                nc.vector.tensor_single_scalar(him[0:118], gw(G), HIGH,
                                               AluOpType.is_gt)
                nc.vector.scalar_tensor_tensor(midm[0:118], gw(G), LOW,
                                               him[0:118], AluOpType.is_ge,
                                               AluOpType.is_gt)
            nc.vector.tensor_tensor(hi[0:118, w], sel, him[0:118],
                                    AluOpType.mult)
            (nc.gpsimd if tail else nc.vector).tensor_tensor(
                mid[0:118, w], midm[0:118], sel, AluOpType.mult)
            ev.tensor_tensor(rs2[0:118, w], hi[0:118, w, :, 0:RB],
                             hi[0:118, w, :, 2:2 + RB], AluOpType.add)
            ev.tensor_tensor(rs2[0:118, w], rs2[0:118, w],
                             hi[0:118, w, :, 1:1 + RB], AluOpType.add)

        def phase3pair():
            # batched phase 3 for the last two chunks (w=0,1): halves the
            # serial link count of the drain chain
            rsp = scr.tile([128, 2, B, RB], bf16, tag="rspP")
            rsm = scr.tile([128, 2, B, RB], bf16, tag="rsmP")
            nc.sync.dma_start(rsp[0:117], rs2[1:118, 0:2])
            nc.sync.dma_start(rsp[117:118], rs2[0:1, 1:3])
            nc.sync.dma_start(rsm[1:118], rs2[0:117, 0:2])
            nc.sync.dma_start(rsm[0:1, 1], rs2[117:118, 0])
            s33 = scr.tile([128, 2, B, RB], bf16, tag="s33P")
            nc.vector.tensor_tensor(s33[0:118], rsp[0:118], rsm[0:118],
                                    AluOpType.add)
            nc.vector.tensor_tensor(s33[0:118], s33[0:118], rs2[0:118, 0:2],
                                    AluOpType.add)
            hic = hi[0:118, 0:2, :, 1:1 + RB]
            cond = scr.tile([128, 2, B, RB], bf16, tag="condP")
            outw = scr.tile([128, 2, B, RB], bf16, tag="outwP")
            nc.vector.tensor_tensor(cond[0:118], s33[0:118], hic,
                                    AluOpType.is_gt)
            nc.vector.tensor_tensor(cond[0:118], cond[0:118],
                                    mid[0:118, 0:2, :, 1:1 + RB],
                                    AluOpType.mult)
            nc.vector.tensor_tensor(outw[0:118], cond[0:118], hic,
                                    AluOpType.max)
            nc.sync.dma_start(o_d[:, 0:2], outw[0:118])

        # ---- phase 3 (per chunk): column-shifted rowsums + combine ----
        def phase3c(w, tail=False):
            ev = nc.vector if tail else nc.gpsimd
            rsp = scr.tile([128, B, RB], bf16, tag="rsp")
            nc.sync.dma_start(rsp[0:117], rs2[1:118, w])
            if w + 1 < NW:
                nc.sync.dma_start(rsp[117:118], rs2[0:1, w + 1])
            rsm = scr.tile([128, B, RB], bf16, tag="rsm")
            nc.sync.dma_start(rsm[1:118], rs2[0:117, w])
            if w > 0:
                nc.sync.dma_start(rsm[0:1], rs2[117:118, w - 1])
            s33 = scr.tile([128, B, RB], bf16, tag="s33")
            ev.tensor_tensor(s33[0:118], rsp[0:118], rsm[0:118],
                             AluOpType.add)
            ev.tensor_tensor(s33[0:118], s33[0:118], rs2[0:118, w],
                             AluOpType.add)
            hic = hi[0:118, w, :, 1:1 + RB]
            cond = scr.tile([128, B, RB], bf16, tag="rsp2")
            outw = scr.tile([128, B, RB], bf16, tag="rsm2")
            nc.vector.tensor_tensor(cond[0:118], s33[0:118], hic,
                                    AluOpType.is_gt)
            ev.tensor_tensor(cond[0:118], cond[0:118],
                             mid[0:118, w, :, 1:1 + RB], AluOpType.mult)
            nc.vector.tensor_tensor(outw[0:118], cond[0:118], hic,
                                    AluOpType.max)
            nc.sync.dma_start(o_d[:, w], outw[0:118])


        # descending chunk order: every phase2c/phase3c boundary dependency
        # (G[w+1], rs2[w+1]) then points to an already-finished chunk, so no
        # stage waits an extra chunk for its right-neighbor boundary column.
        order = list(range(NW - 1, -1, -1))
        gx = {}
        pre2 = {}
        for i, w in enumerate(order):
            xw = xw_next
            if i + 1 < NW:
                xw_next = _load_x(order[i + 1])
            gx[w] = stepA(w, xw)
            if i >= 1:
                stepB(order[i - 1], gx.pop(order[i - 1]))
            if i >= 2:
                wq = order[i - 2]
                phase2c(wq, pre2.pop(wq))
            if i >= 1:
                wp = order[i - 1]
                pre2[wp] = phase2dma(wp)
            if i >= 3:
                phase3c(order[i - 3])
        stepB(order[-1], gx.pop(order[-1]), tail=True)
        phase2c(order[-2], pre2.pop(order[-2]), tail=True)
        pre2[order[-1]] = phase2dma(order[-1])
        phase3c(order[-3], tail=True)
        phase2c(order[-1], pre2.pop(order[-1]), tail=True)
        phase3pair()

    return nc
